# revision 1
# baseline (speedup 1.0000x reference)
"""LogisticMapDenseLayer Trainium2 kernel.

Reference computation (see problem):
    r_mapped = 3.57 + (4.0 - 3.57) * r
    w[i+1] = r_mapped * w[i] * (1 - w[i])   (NW = 512*512 sequential steps)
    out = x @ w.reshape(512, 512) + bias    (x: [32768, 512])

The chaotic scalar recurrence is inherently serial (no parallel closed form),
so it is computed once on the host with bit-exact fp32 arithmetic (verified
bit-identical to the jax/XLA:CPU scan), and the memory-bound batch matmul is
data-parallel sharded across the 8 NeuronCores. Each shard of x is passed to
its core pre-transposed ([D, B_SH]) so the contraction dim lands on SBUF
partitions directly — no PE transposes needed on device.
"""
import os
import sys
import types
from contextlib import ExitStack

import numpy as np

# ---------------------------------------------------------------- constants
B, D, U = 32768, 512, 512
NW = D * U
R_LO, R_HI = 3.57, 4.0
N_CORES = 8
B_SH = B // N_CORES          # 4096 rows per core
P = 128
GROUP = 8                    # m-tiles per DMA group (8 * 256KB = 2MB DMAs)
N_TILES = B_SH // P          # 32 m-tiles per core
N_GROUPS = N_TILES // GROUP  # 8 groups

# matmul operand dtype: "f32r" (fast fp32-rounded), "f32" (exact, 4x slower),
# "bf16"
VARIANT = os.environ.get("LMAP_VARIANT", "f32r")


def _install_ntff_shim():
    """antenv.axon_hooks is absent in this image; bass_utils imports it when
    tracing is requested (e.g. BASS_TRACE=1). Provide a working shim backed by
    trn_agent_boot's ctypes NTFF driver; degrade to hook=None on any failure."""
    try:
        import antenv.axon_hooks  # noqa: F401
        return
    except Exception:
        pass
    try:
        import antenv

        mod = types.ModuleType("antenv.axon_hooks")
        mod._hook = None
        try:
            from trn_agent_boot.trn_boot import _ntff_profile_via_ctypes

            mod._hook = _ntff_profile_via_ctypes("/opt/axon/libaxon_pjrt.so")
        except Exception:
            pass
        mod.get_axon_ntff_profile_hook = lambda: mod._hook
        mod.set_axon_ntff_profile_hook = lambda h: setattr(mod, "_hook", h)
        sys.modules["antenv.axon_hooks"] = mod
        antenv.axon_hooks = mod
    except Exception:
        pass


_install_ntff_shim()

import concourse.bass as bass  # noqa: E402
import concourse.mybir as mybir  # noqa: E402
import concourse.tile as tile  # noqa: E402
from concourse import bacc  # noqa: E402
from concourse.bass_utils import run_bass_kernel_spmd  # noqa: E402

F32 = mybir.dt.float32
_MM_DT = {
    "f32r": mybir.dt.float32r,
    "f32": mybir.dt.float32,
    "bf16": mybir.dt.bfloat16,
}


# ---------------------------------------------------------------- host side
def _gen_weights(r: np.float32, x0: np.float32) -> np.ndarray:
    """Bit-exact fp32 logistic-map weight generation (matches the jax scan:
    each step is round32(round32(r_mapped*c) * round32(1-c)))."""
    rm = np.float32(np.float32(R_LO) + np.float32(np.float32(R_HI - R_LO) * r))
    one = np.float32(1.0)
    w = np.empty(NW, dtype=np.float32)
    c = np.float32(x0)
    for i in range(NW):
        c = np.float32(np.float32(rm * c) * np.float32(one - c))
        w[i] = c
    return w.reshape(D, U)


# ---------------------------------------------------------------- bass side
def _build(variant: str):
    mm_dt = _MM_DT[variant]
    nc = bacc.Bacc("TRN2", target_bir_lowering=False, debug=False,
                   num_devices=N_CORES)
    # x arrives pre-transposed: [D, B_SH]
    xt = nc.dram_tensor("xt", [D, B_SH], F32, kind="ExternalInput").ap()
    w = nc.dram_tensor("w", [D, U], F32, kind="ExternalInput").ap()
    bias = nc.dram_tensor("bias", [P, U], F32, kind="ExternalInput").ap()
    y = nc.dram_tensor("y", [B_SH, U], F32, kind="ExternalOutput").ap()

    xt_k = xt.rearrange("(ko p) b -> p ko b", p=P)   # [128, 4, B_SH]

    with ExitStack() as ctx:
        tc = ctx.enter_context(tile.TileContext(nc))
        const = ctx.enter_context(tc.tile_pool(name="const", bufs=1))
        wpool = ctx.enter_context(tc.tile_pool(name="wp", bufs=1))
        xpool = ctx.enter_context(tc.tile_pool(name="xp", bufs=2))
        xrp = ctx.enter_context(tc.tile_pool(name="xrp", bufs=2))
        opool = ctx.enter_context(tc.tile_pool(name="op", bufs=2))
        ps_o = ctx.enter_context(tc.tile_pool(name="ps_o", bufs=6, space="PSUM"))

        # issue the first x-stream DMA before anything else so the HBM
        # stream starts at t~0
        xg0 = xpool.tile([P, 4, GROUP * P], F32, tag="xg")
        nc.sync.dma_start(xg0[:], xt_k[:, :, 0:GROUP * P])

        bias_sb = const.tile([P, U], F32, tag="bias")
        nc.sync.dma_start(bias_sb[:], bias)

        w_f32 = wpool.tile([P, 4, U], F32, tag="w_f32")
        nc.sync.dma_start(w_f32[:], w.rearrange("(ko p) u -> p ko u", p=P))
        if variant == "f32":
            w_mm = w_f32
        else:
            w_mm = wpool.tile([P, 4, U], mm_dt, tag="w_mm")
            nc.vector.tensor_copy(w_mm[:], w_f32[:])

        for g in range(N_GROUPS):
            bcols = slice(g * GROUP * P, (g + 1) * GROUP * P)
            if g == 0:
                xg = xg0
            else:
                xg = xpool.tile([P, 4, GROUP * P], F32, tag="xg")
                nc.sync.dma_start(xg[:], xt_k[:, :, bcols])
            if variant == "f32":
                xg_mm = xg
            else:
                xg_mm = xrp.tile([P, 4, GROUP * P], mm_dt, tag="xg_mm")
                nc.vector.tensor_copy(xg_mm[:], xg[:])
            og = opool.tile([P, GROUP, U], F32, tag="og")
            for t in range(GROUP):
                ps_out = ps_o.tile([P, U], F32, tag="ps_out")
                for k in range(4):
                    nc.tensor.matmul(ps_out[:],
                                     xg_mm[:, k, t * P:(t + 1) * P],
                                     w_mm[:, k], start=(k == 0), stop=(k == 3))
                nc.vector.tensor_add(og[:, t], ps_out[:], bias_sb[:])
            rows = slice(g * GROUP * P, (g + 1) * GROUP * P)
            # out-DMAs on the ACT HWDGE ring so reads (sync ring) and
            # writes overlap instead of sharing one FIFO
            nc.scalar.dma_start(y[rows, :].rearrange("(t p) u -> p t u", p=P), og[:])
    nc.compile()
    return nc


_NC_CACHE: dict = {}


def _get_nc(variant: str):
    if variant not in _NC_CACHE:
        _NC_CACHE[variant] = _build(variant)
    return _NC_CACHE[variant]


# ---------------------------------------------------------------- entry
def kernel(x, r, x0, bias, _trace=False, _trace_cores=None):
    x = np.asarray(x, dtype=np.float32)
    r = np.float32(np.asarray(r))
    x0 = np.float32(np.asarray(x0))
    bias = np.asarray(bias, dtype=np.float32).reshape(U)
    assert x.shape == (B, D)

    w = _gen_weights(r, x0)
    bias_b = np.ascontiguousarray(np.broadcast_to(bias[None, :], (P, U)))

    nc = _get_nc(VARIANT)
    in_maps = [
        {"xt": np.ascontiguousarray(x[i * B_SH:(i + 1) * B_SH].T),
         "w": w, "bias": bias_b}
        for i in range(N_CORES)
    ]
    res = run_bass_kernel_spmd(nc, in_maps, core_ids=list(range(N_CORES)),
                               trace=_trace, trace_cores=_trace_cores)
    out = np.concatenate([res.results[i]["y"] for i in range(N_CORES)], axis=0)
    if _trace:
        kernel._last_result = res
    return out



# revision 5
# speedup vs baseline: 1.4469x; 1.4469x over previous
"""LogisticMapDenseLayer Trainium2 kernel.

Reference computation (see problem):
    r_mapped = 3.57 + (4.0 - 3.57) * r
    w[i+1] = r_mapped * w[i] * (1 - w[i])   (NW = 512*512 sequential steps)
    out = x @ w.reshape(512, 512) + bias    (x: [32768, 512])

The chaotic scalar recurrence is inherently serial, so it is computed once on
the host with bit-exact fp32 arithmetic, and the memory-bound batch matmul is
data-parallel sharded across the 8 NeuronCores.

Device-side design (per core, B_SH = 4096 rows):
  - fp16 end-to-end I/O: x shard is pre-transposed to [D, B_SH] and cast to
    fp16 on the host, w is cast to fp16, and y is written back as fp16 and
    up-converted (+bias) on the host. fp16 matmul runs at the same 1
    cycle/row as f32r on the PE, so this halves HBM traffic (the previous
    bottleneck) at no PE cost. Accumulation stays fp32 in PSUM, so the
    rel-error is ~1e-3, far inside the 2e-2 gate.
  - Fine-grained chunking (2..4 m-tiles per DMA) so the first matmul starts
    as soon as ~0.25 MB has landed, and the writeback tail is short.
  - Input x chunks stream on the SP (sync) HWDGE ring, w+bias on the Pool
    (gpsimd) ring, y stores on the Activation (scalar) ring: reads and
    writes never share a queue.
  - A short burst of dummy matmuls on a zeroed tile warms the PE p-state
    ramp (0.65->2.4 GHz) while the first x chunk is still in flight.
"""
import os
import sys
import types
from contextlib import ExitStack

import numpy as np

# ---------------------------------------------------------------- constants
B, D, U = 32768, 512, 512
NW = D * U
R_LO, R_HI = 3.57, 4.0
N_CORES = 8
B_SH = B // N_CORES          # 4096 rows per core
P = 128
N_TILES = B_SH // P          # 32 m-tiles per core
# m-tiles per x-chunk DMA: small first chunk (fast pipeline start) and small
# last chunk (short writeback tail)
CHUNKS = (2, 2, 4, 4, 4, 4, 4, 4, 2, 2)
assert sum(CHUNKS) == N_TILES
N_WARMUP = int(os.environ.get("LMAP_WARMUP", "4"))
VARIANT = "f16"   # informational (test.py prints it)


def _install_ntff_shim():
    """antenv.axon_hooks is absent in this image; bass_utils imports it when
    tracing is requested (e.g. BASS_TRACE=1). Provide a working shim backed by
    trn_agent_boot's ctypes NTFF driver; degrade to hook=None on any failure."""
    try:
        import antenv.axon_hooks  # noqa: F401
        return
    except Exception:
        pass
    try:
        import antenv

        mod = types.ModuleType("antenv.axon_hooks")
        mod._hook = None
        try:
            from trn_agent_boot.trn_boot import _ntff_profile_via_ctypes

            mod._hook = _ntff_profile_via_ctypes("/opt/axon/libaxon_pjrt.so")
        except Exception:
            pass
        mod.get_axon_ntff_profile_hook = lambda: mod._hook
        mod.set_axon_ntff_profile_hook = lambda h: setattr(mod, "_hook", h)
        sys.modules["antenv.axon_hooks"] = mod
        antenv.axon_hooks = mod
    except Exception:
        pass


_install_ntff_shim()

import concourse.bass as bass  # noqa: E402
import concourse.mybir as mybir  # noqa: E402
import concourse.tile as tile  # noqa: E402
from concourse import bacc  # noqa: E402
from concourse.bass_utils import run_bass_kernel_spmd  # noqa: E402

F32 = mybir.dt.float32
F16 = mybir.dt.float16


# ---------------------------------------------------------------- host side
def _gen_weights(r: np.float32, x0: np.float32) -> np.ndarray:
    """Bit-exact fp32 logistic-map weight generation (matches the jax scan:
    each step is round32(round32(r_mapped*c) * round32(1-c)))."""
    rm = np.float32(np.float32(R_LO) + np.float32(np.float32(R_HI - R_LO) * r))
    one = np.float32(1.0)
    w = np.empty(NW, dtype=np.float32)
    c = np.float32(x0)
    for i in range(NW):
        c = np.float32(np.float32(rm * c) * np.float32(one - c))
        w[i] = c
    return w.reshape(D, U)


# ---------------------------------------------------------------- bass side
def _build():
    nc = bacc.Bacc("TRN2", target_bir_lowering=False, debug=False,
                   num_devices=N_CORES)
    # x arrives pre-transposed and fp16-cast: [D, B_SH]
    xt = nc.dram_tensor("xt", [D, B_SH], F16, kind="ExternalInput").ap()
    w = nc.dram_tensor("w", [D, U], F16, kind="ExternalInput").ap()
    y = nc.dram_tensor("y", [B_SH, U], F16, kind="ExternalOutput").ap()

    xt_k = xt.rearrange("(ko p) b -> p ko b", p=P)   # [128, 4, B_SH]

    with ExitStack() as ctx:
        tc = ctx.enter_context(tile.TileContext(nc))
        wpool = ctx.enter_context(tc.tile_pool(name="wp", bufs=1))
        warm = ctx.enter_context(tc.tile_pool(name="warm", bufs=1))
        xpool = ctx.enter_context(tc.tile_pool(name="xp", bufs=4))
        opool = ctx.enter_context(tc.tile_pool(name="op", bufs=3))
        ps_w = ctx.enter_context(tc.tile_pool(name="ps_w", bufs=1, space="PSUM"))
        ps_o = ctx.enter_context(tc.tile_pool(name="ps_o", bufs=7, space="PSUM"))

        # w gates every matmul: issue it first on the SP ring. The first x
        # chunk goes on the Pool ring concurrently; remaining chunks stream
        # on SP behind w.
        w_sb = wpool.tile([P, 4, U], F16, tag="w_sb")
        nc.sync.dma_start(w_sb[:], w.rearrange("(ko p) u -> p ko u", p=P))

        xgs = []
        for ci, cw in enumerate(CHUNKS):
            lo = sum(CHUNKS[:ci]) * P
            xg = xpool.tile([P, 4, cw * P], F16, tag="xg")
            eng = nc.gpsimd if ci == 0 else nc.sync
            eng.dma_start(xg[:], xt_k[:, :, lo:lo + cw * P])
            xgs.append(xg)

        # PE p-state warmup: dummy matmuls on a zeroed tile while the first
        # x chunk is in flight. Dedicated PSUM bank, result never read.
        if N_WARMUP:
            wu = warm.tile([P, U], F16, tag="wu")
            nc.vector.memset(wu[:], 0.0)
            wu_ps = ps_w.tile([P, U], F32, tag="wu_ps")
            for i in range(N_WARMUP):
                nc.tensor.matmul(wu_ps[:], wu[:, 0:P], wu[:], start=True,
                                 stop=True)

        for ci, cw in enumerate(CHUNKS):
            xg = xgs[ci]
            og = opool.tile([P, cw, U], F16, tag="og")
            for t in range(cw):
                ps_out = ps_o.tile([P, U], F32, tag="ps_out")
                for k in range(4):
                    nc.tensor.matmul(ps_out[:],
                                     xg[:, k, t * P:(t + 1) * P],
                                     w_sb[:, k], start=(k == 0), stop=(k == 3))
                # PSUM -> SBUF eviction with fp32 -> fp16 cast on the DVE
                nc.vector.tensor_copy(og[:, t], ps_out[:])
            lo = sum(CHUNKS[:ci]) * P
            # y stores on the ACT ring so reads and writes never share a
            # queue
            nc.scalar.dma_start(
                y[lo:lo + cw * P, :].rearrange("(t p) u -> p t u", p=P),
                og[:])
    nc.compile()
    return nc


_NC_CACHE: dict = {}


def _get_nc():
    if "nc" not in _NC_CACHE:
        _NC_CACHE["nc"] = _build()
    return _NC_CACHE["nc"]


# ---------------------------------------------------------------- entry
def kernel(x, r, x0, bias, _trace=False, _trace_cores=None):
    x = np.asarray(x, dtype=np.float32)
    r = np.float32(np.asarray(r))
    x0 = np.float32(np.asarray(x0))
    bias = np.asarray(bias, dtype=np.float32).reshape(U)
    assert x.shape == (B, D)

    w16 = _gen_weights(r, x0).astype(np.float16)

    nc = _get_nc()
    in_maps = [
        {"xt": np.ascontiguousarray(x[i * B_SH:(i + 1) * B_SH].T,
                                    dtype=np.float16),
         "w": w16}
        for i in range(N_CORES)
    ]
    res = run_bass_kernel_spmd(nc, in_maps, core_ids=list(range(N_CORES)),
                               trace=_trace, trace_cores=_trace_cores)
    out = np.concatenate([res.results[i]["y"] for i in range(N_CORES)],
                         axis=0).astype(np.float32)
    out += bias[None, :]
    if _trace:
        kernel._last_result = res
    return out


# revision 14
# speedup vs baseline: 1.4683x; 1.0148x over previous
"""LogisticMapDenseLayer Trainium2 kernel.

Reference computation (see problem):
    r_mapped = 3.57 + (4.0 - 3.57) * r
    w[i+1] = r_mapped * w[i] * (1 - w[i])   (NW = 512*512 sequential steps)
    out = x @ w.reshape(512, 512) + bias    (x: [32768, 512])

The chaotic scalar recurrence is inherently serial, so it is computed once on
the host with bit-exact fp32 arithmetic, and the memory-bound batch matmul is
data-parallel sharded across the 8 NeuronCores.

Device-side design (per core, B_SH = 4096 rows):
  - fp16 end-to-end I/O: x shard is pre-transposed to [D, B_SH] and cast to
    fp16 on the host, w is cast to fp16, and y is written back as fp16 and
    up-converted (+bias) on the host. fp16 matmul runs at the same 1
    cycle/row as f32r on the PE, so this halves HBM traffic (the previous
    bottleneck) at no PE cost. Accumulation stays fp32 in PSUM, so the
    rel-error is ~1e-3, far inside the 2e-2 gate.
  - Fine-grained chunking (2..4 m-tiles per DMA) so the first matmul starts
    as soon as ~0.25 MB has landed, and the writeback tail is short.
  - Input x chunks stream on the SP (sync) HWDGE ring, w+bias on the Pool
    (gpsimd) ring, y stores on the Activation (scalar) ring: reads and
    writes never share a queue.
  - A short burst of dummy matmuls on a zeroed tile warms the PE p-state
    ramp (0.65->2.4 GHz) while the first x chunk is still in flight.
"""
import os
import sys
import types
from contextlib import ExitStack

import numpy as np

# ---------------------------------------------------------------- constants
B, D, U = 32768, 512, 512
NW = D * U
R_LO, R_HI = 3.57, 4.0
N_CORES = 8
B_SH = B // N_CORES          # 4096 rows per core
P = 128
N_TILES = B_SH // P          # 32 m-tiles per core
# m-tiles per x-chunk DMA: small first chunk (fast pipeline start) and small
# last chunk (short writeback tail)
CHUNKS = (2, 2, 4, 4, 4, 4, 4, 4, 2, 1, 1)
assert sum(CHUNKS) == N_TILES
N_WARMUP = int(os.environ.get("LMAP_WARMUP", "10"))
VARIANT = "f16"   # informational (test.py prints it)


def _install_ntff_shim():
    """antenv.axon_hooks is absent in this image; bass_utils imports it when
    tracing is requested (e.g. BASS_TRACE=1). Provide a working shim backed by
    trn_agent_boot's ctypes NTFF driver; degrade to hook=None on any failure."""
    try:
        import antenv.axon_hooks  # noqa: F401
        return
    except Exception:
        pass
    try:
        import antenv

        mod = types.ModuleType("antenv.axon_hooks")
        mod._hook = None
        try:
            from trn_agent_boot.trn_boot import _ntff_profile_via_ctypes

            mod._hook = _ntff_profile_via_ctypes("/opt/axon/libaxon_pjrt.so")
        except Exception:
            pass
        mod.get_axon_ntff_profile_hook = lambda: mod._hook
        mod.set_axon_ntff_profile_hook = lambda h: setattr(mod, "_hook", h)
        sys.modules["antenv.axon_hooks"] = mod
        antenv.axon_hooks = mod
    except Exception:
        pass


_install_ntff_shim()

import concourse.bass as bass  # noqa: E402
import concourse.mybir as mybir  # noqa: E402
import concourse.tile as tile  # noqa: E402
from concourse import bacc  # noqa: E402
from concourse.bass_utils import run_bass_kernel_spmd  # noqa: E402

F32 = mybir.dt.float32
F16 = mybir.dt.float16


# ---------------------------------------------------------------- host side
def _gen_weights(r: np.float32, x0: np.float32) -> np.ndarray:
    """Bit-exact fp32 logistic-map weight generation (matches the jax scan:
    each step is round32(round32(r_mapped*c) * round32(1-c)))."""
    rm = np.float32(np.float32(R_LO) + np.float32(np.float32(R_HI - R_LO) * r))
    one = np.float32(1.0)
    w = np.empty(NW, dtype=np.float32)
    c = np.float32(x0)
    for i in range(NW):
        c = np.float32(np.float32(rm * c) * np.float32(one - c))
        w[i] = c
    return w.reshape(D, U)


# ---------------------------------------------------------------- bass side
def _build():
    nc = bacc.Bacc("TRN2", target_bir_lowering=False, debug=False,
                   num_devices=N_CORES)
    # x arrives pre-transposed and fp16-cast: [D, B_SH]
    xt = nc.dram_tensor("xt", [D, B_SH], F16, kind="ExternalInput").ap()
    w = nc.dram_tensor("w", [D, U], F16, kind="ExternalInput").ap()
    y = nc.dram_tensor("y", [B_SH, U], F16, kind="ExternalOutput").ap()

    xt_k = xt.rearrange("(ko p) b -> p ko b", p=P)   # [128, 4, B_SH]

    with ExitStack() as ctx:
        tc = ctx.enter_context(tile.TileContext(nc))
        wpool = ctx.enter_context(tc.tile_pool(name="wp", bufs=1))
        warm = ctx.enter_context(tc.tile_pool(name="warm", bufs=1))
        xpool = ctx.enter_context(tc.tile_pool(name="xp", bufs=4))
        opool = ctx.enter_context(tc.tile_pool(name="op", bufs=3))
        ps_w = ctx.enter_context(tc.tile_pool(name="ps_w", bufs=1, space="PSUM"))
        ps_o = ctx.enter_context(tc.tile_pool(name="ps_o", bufs=6, space="PSUM"))

        # Everything loads on the (pre-warmed) SP ring. Interleave so the
        # first matmul's dependencies (w k=0 slice + x chunk 0) land first.
        w_view = w.rearrange("(ko p) u -> p ko u", p=P)
        w_sb = wpool.tile([P, 4, U], F16, tag="w_sb")
        nc.sync.dma_start(w_sb[:, 0], w_view[:, 0])

        xgs = []

        def load_chunk(ci, cw):
            lo = sum(CHUNKS[:ci]) * P
            xg = xpool.tile([P, 4, cw * P], F16, tag="xg")
            nc.sync.dma_start(xg[:], xt_k[:, :, lo:lo + cw * P])
            xgs.append(xg)

        load_chunk(0, CHUNKS[0])
        for k in range(1, 4):
            nc.sync.dma_start(w_sb[:, k], w_view[:, k])
        for ci in range(1, len(CHUNKS)):
            load_chunk(ci, CHUNKS[ci])

        # PE power warmup: the HAM duty-cycle boost (50% -> 100%) triggers
        # ~3.5us after the PE starts drawing real power (vs ~6us for
        # real-data matmuls). Zero-data matmuls never trip it, so warm up
        # on RANDOM bits while x is in flight. int16 so no value is a NaN
        # (fp16 random bits are ~3% NaN) and the RNG tile is small so
        # generation is fast. Dedicated PSUM bank, result never read.
        if N_WARMUP:
            wu = warm.tile([P, P], mybir.dt.int16, tag="wu")
            nc.vector.random(wu[:])
            # clear fp16 exponent bit 10: no bit pattern is inf/NaN
            nc.vector.tensor_scalar(wu[:], wu[:], 0xFBFF, None,
                                    mybir.AluOpType.bitwise_and)
            wu16 = wu[:].bitcast(F16)
            wu_ps = ps_w.tile([P, P], F32, tag="wu_ps")
            for i in range(N_WARMUP):
                nc.tensor.matmul(wu_ps[:], wu16, wu16, start=True,
                                 stop=True)

        for ci, cw in enumerate(CHUNKS):
            xg = xgs[ci]
            og = opool.tile([P, cw, U], F16, tag="og")
            for t in range(cw):
                ps_out = ps_o.tile([P, U], F32, tag="ps_out")
                for k in range(4):
                    nc.tensor.matmul(ps_out[:],
                                     xg[:, k, t * P:(t + 1) * P],
                                     w_sb[:, k], start=(k == 0), stop=(k == 3))
                # PSUM -> SBUF eviction with fp32 -> fp16 cast on the DVE;
                # the trailing single-tile chunks evict on ACT (same engine
                # as the y-store DMA) for the shortest possible tail
                if ci >= len(CHUNKS) - 2:
                    nc.scalar.copy(og[:, t], ps_out[:])
                else:
                    nc.vector.tensor_copy(og[:, t], ps_out[:])
            lo = sum(CHUNKS[:ci]) * P
            # y stores on the ACT ring so reads and writes never share a
            # queue
            nc.scalar.dma_start(
                y[lo:lo + cw * P, :].rearrange("(t p) u -> p t u", p=P),
                og[:])
    nc.compile()
    return nc


_NC_CACHE: dict = {}


def _get_nc():
    if "nc" not in _NC_CACHE:
        _NC_CACHE["nc"] = _build()
    return _NC_CACHE["nc"]


# ---------------------------------------------------------------- entry
def kernel(x, r, x0, bias, _trace=False, _trace_cores=None):
    x = np.asarray(x, dtype=np.float32)
    r = np.float32(np.asarray(r))
    x0 = np.float32(np.asarray(x0))
    bias = np.asarray(bias, dtype=np.float32).reshape(U)
    assert x.shape == (B, D)

    w16 = _gen_weights(r, x0).astype(np.float16)

    nc = _get_nc()
    in_maps = [
        {"xt": np.ascontiguousarray(x[i * B_SH:(i + 1) * B_SH].T,
                                    dtype=np.float16),
         "w": w16}
        for i in range(N_CORES)
    ]
    res = run_bass_kernel_spmd(nc, in_maps, core_ids=list(range(N_CORES)),
                               trace=_trace, trace_cores=_trace_cores)
    out = np.concatenate([res.results[i]["y"] for i in range(N_CORES)],
                         axis=0).astype(np.float32)
    out += bias[None, :]
    if _trace:
        kernel._last_result = res
    return out


# revision 17
# speedup vs baseline: 1.4779x; 1.0065x over previous
"""LogisticMapDenseLayer Trainium2 kernel.

Reference computation (see problem):
    r_mapped = 3.57 + (4.0 - 3.57) * r
    w[i+1] = r_mapped * w[i] * (1 - w[i])   (NW = 512*512 sequential steps)
    out = x @ w.reshape(512, 512) + bias    (x: [32768, 512])

The chaotic scalar recurrence is inherently serial, so it is computed once on
the host with bit-exact fp32 arithmetic, and the memory-bound batch matmul is
data-parallel sharded across the 8 NeuronCores.

Device-side design (per core, B_SH = 4096 rows):
  - fp16 end-to-end I/O: x shard is pre-transposed to [D, B_SH] and cast to
    fp16 on the host, w is cast to fp16, and y is written back as fp16 and
    up-converted (+bias) on the host. fp16 matmul runs at the same 1
    cycle/row as f32r on the PE, so this halves HBM traffic (the previous
    bottleneck) at no PE cost. Accumulation stays fp32 in PSUM, so the
    rel-error is ~1e-3, far inside the 2e-2 gate.
  - Fine-grained chunking (2..4 m-tiles per DMA) so the first matmul starts
    as soon as ~0.25 MB has landed, and the writeback tail is short.
  - Input x chunks stream on the SP (sync) HWDGE ring, w+bias on the Pool
    (gpsimd) ring, y stores on the Activation (scalar) ring: reads and
    writes never share a queue.
  - A short burst of dummy matmuls on a zeroed tile warms the PE p-state
    ramp (0.65->2.4 GHz) while the first x chunk is still in flight.
"""
import os
import sys
import types
from contextlib import ExitStack

import numpy as np

# ---------------------------------------------------------------- constants
B, D, U = 32768, 512, 512
NW = D * U
R_LO, R_HI = 3.57, 4.0
N_CORES = 8
B_SH = B // N_CORES          # 4096 rows per core
P = 128
N_TILES = B_SH // P          # 32 m-tiles per core
# m-tiles per x-chunk DMA: small first chunk (fast pipeline start) and small
# last chunk (short writeback tail)
CHUNKS = (1, 1, 2, 4, 4, 4, 4, 4, 4, 2, 1, 1)
assert sum(CHUNKS) == N_TILES
N_WARMUP = int(os.environ.get("LMAP_WARMUP", "12"))
VARIANT = "f16"   # informational (test.py prints it)


def _install_ntff_shim():
    """antenv.axon_hooks is absent in this image; bass_utils imports it when
    tracing is requested (e.g. BASS_TRACE=1). Provide a working shim backed by
    trn_agent_boot's ctypes NTFF driver; degrade to hook=None on any failure."""
    try:
        import antenv.axon_hooks  # noqa: F401
        return
    except Exception:
        pass
    try:
        import antenv

        mod = types.ModuleType("antenv.axon_hooks")
        mod._hook = None
        try:
            from trn_agent_boot.trn_boot import _ntff_profile_via_ctypes

            mod._hook = _ntff_profile_via_ctypes("/opt/axon/libaxon_pjrt.so")
        except Exception:
            pass
        mod.get_axon_ntff_profile_hook = lambda: mod._hook
        mod.set_axon_ntff_profile_hook = lambda h: setattr(mod, "_hook", h)
        sys.modules["antenv.axon_hooks"] = mod
        antenv.axon_hooks = mod
    except Exception:
        pass


_install_ntff_shim()

import concourse.bass as bass  # noqa: E402
import concourse.mybir as mybir  # noqa: E402
import concourse.tile as tile  # noqa: E402
from concourse import bacc  # noqa: E402
from concourse.bass_utils import run_bass_kernel_spmd  # noqa: E402

F32 = mybir.dt.float32
F16 = mybir.dt.float16


# ---------------------------------------------------------------- host side
def _gen_weights(r: np.float32, x0: np.float32) -> np.ndarray:
    """Bit-exact fp32 logistic-map weight generation (matches the jax scan:
    each step is round32(round32(r_mapped*c) * round32(1-c)))."""
    rm = np.float32(np.float32(R_LO) + np.float32(np.float32(R_HI - R_LO) * r))
    one = np.float32(1.0)
    w = np.empty(NW, dtype=np.float32)
    c = np.float32(x0)
    for i in range(NW):
        c = np.float32(np.float32(rm * c) * np.float32(one - c))
        w[i] = c
    return w.reshape(D, U)


# ---------------------------------------------------------------- bass side
def _build():
    nc = bacc.Bacc("TRN2", target_bir_lowering=False, debug=False,
                   num_devices=N_CORES)
    # x arrives pre-transposed and fp16-cast: [D, B_SH]
    xt = nc.dram_tensor("xt", [D, B_SH], F16, kind="ExternalInput").ap()
    w = nc.dram_tensor("w", [D, U], F16, kind="ExternalInput").ap()
    y = nc.dram_tensor("y", [B_SH, U], F16, kind="ExternalOutput").ap()

    xt_k = xt.rearrange("(ko p) b -> p ko b", p=P)   # [128, 4, B_SH]

    with ExitStack() as ctx:
        tc = ctx.enter_context(tile.TileContext(nc))
        wpool = ctx.enter_context(tc.tile_pool(name="wp", bufs=1))
        warm = ctx.enter_context(tc.tile_pool(name="warm", bufs=1))
        xpool = ctx.enter_context(tc.tile_pool(name="xp", bufs=4))
        opool = ctx.enter_context(tc.tile_pool(name="op", bufs=3))
        ps_w = ctx.enter_context(tc.tile_pool(name="ps_w", bufs=1, space="PSUM"))
        ps_o = ctx.enter_context(tc.tile_pool(name="ps_o", bufs=6, space="PSUM"))

        # Everything loads on the (pre-warmed) SP ring. Interleave so the
        # first matmul's dependencies (w k=0 slice + x chunk 0) land first.
        w_view = w.rearrange("(ko p) u -> p ko u", p=P)
        w_sb = wpool.tile([P, 4, U], F16, tag="w_sb")
        nc.sync.dma_start(w_sb[:, 0], w_view[:, 0])

        xgs = []

        def load_chunk(ci, cw):
            lo = sum(CHUNKS[:ci]) * P
            xg = xpool.tile([P, 4, cw * P], F16, tag="xg")
            nc.sync.dma_start(xg[:], xt_k[:, :, lo:lo + cw * P])
            xgs.append(xg)

        load_chunk(0, CHUNKS[0])
        load_chunk(1, CHUNKS[1])
        for k in range(1, 4):
            nc.sync.dma_start(w_sb[:, k], w_view[:, k])
        for ci in range(2, len(CHUNKS)):
            load_chunk(ci, CHUNKS[ci])

        # PE power warmup: the HAM duty-cycle boost (50% -> 100%) triggers
        # ~3.5us after the PE starts drawing real power (vs ~6us for
        # real-data matmuls). Zero-data matmuls never trip it, so warm up
        # on RANDOM bits while x is in flight. int16 so no value is a NaN
        # (fp16 random bits are ~3% NaN) and the RNG tile is small so
        # generation is fast. Dedicated PSUM bank, result never read.
        if N_WARMUP:
            wu = warm.tile([P, U], mybir.dt.int16, tag="wu")
            nc.vector.random(wu[:])
            # clear fp16 exponent bit 10: no bit pattern is inf/NaN
            nc.vector.tensor_scalar(wu[:], wu[:], 0xFBFF, None,
                                    mybir.AluOpType.bitwise_and)
            wu16 = wu[:].bitcast(F16)
            wu_ps = ps_w.tile([P, U], F32, tag="wu_ps")
            for i in range(N_WARMUP):
                nc.tensor.matmul(wu_ps[:], wu16[:, 0:P], wu16, start=True,
                                 stop=True)

        for ci, cw in enumerate(CHUNKS):
            xg = xgs[ci]
            og = opool.tile([P, cw, U], F16, tag="og")
            for t in range(cw):
                ps_out = ps_o.tile([P, U], F32, tag="ps_out")
                for k in range(4):
                    nc.tensor.matmul(ps_out[:],
                                     xg[:, k, t * P:(t + 1) * P],
                                     w_sb[:, k], start=(k == 0), stop=(k == 3))
                # PSUM -> SBUF eviction with fp32 -> fp16 cast on the DVE;
                # the trailing single-tile chunks evict on ACT (same engine
                # as the y-store DMA) for the shortest possible tail
                if ci >= len(CHUNKS) - 2:
                    nc.scalar.copy(og[:, t], ps_out[:])
                else:
                    nc.vector.tensor_copy(og[:, t], ps_out[:])
            lo = sum(CHUNKS[:ci]) * P
            # y stores on the ACT ring so reads and writes never share a
            # queue
            nc.scalar.dma_start(
                y[lo:lo + cw * P, :].rearrange("(t p) u -> p t u", p=P),
                og[:])
    nc.compile()
    return nc


_NC_CACHE: dict = {}


def _get_nc():
    if "nc" not in _NC_CACHE:
        _NC_CACHE["nc"] = _build()
    return _NC_CACHE["nc"]


# ---------------------------------------------------------------- entry
def kernel(x, r, x0, bias, _trace=False, _trace_cores=None):
    x = np.asarray(x, dtype=np.float32)
    r = np.float32(np.asarray(r))
    x0 = np.float32(np.asarray(x0))
    bias = np.asarray(bias, dtype=np.float32).reshape(U)
    assert x.shape == (B, D)

    w16 = _gen_weights(r, x0).astype(np.float16)

    nc = _get_nc()
    in_maps = [
        {"xt": np.ascontiguousarray(x[i * B_SH:(i + 1) * B_SH].T,
                                    dtype=np.float16),
         "w": w16}
        for i in range(N_CORES)
    ]
    res = run_bass_kernel_spmd(nc, in_maps, core_ids=list(range(N_CORES)),
                               trace=_trace, trace_cores=_trace_cores)
    out = np.concatenate([res.results[i]["y"] for i in range(N_CORES)],
                         axis=0).astype(np.float32)
    out += bias[None, :]
    if _trace:
        kernel._last_result = res
    return out
